# revision 1
# baseline (speedup 1.0000x reference)
# Multi-head attention (B=2, S=4096, D=768, H=12) on 8 Trainium2 NeuronCores.
#
# Sharding: 24 (batch, head) units -> 3 heads x 1 batch per core.
#   core c: batch b = c // 4, heads h0..h0+2 where h0 = 3 * (c % 4).
# Each core computes q/k/v projections for its heads, attention, and a
# row-parallel partial of the output projection (its 192 columns of the
# concat dimension).  Host sums the 4 partials per batch and adds bo.
#
# Device layout notes:
#   - activations are fed transposed ([D, S]) so the PE contracts over
#     partitions; qT/kT stay transposed ([64, S]) which is exactly the
#     layout both QK^T and the PE-side rowsum want.
#   - softmax skips max-subtraction (scores ~ N(0,1) by construction;
#     exp stays in fp32 range), so softmax is: exp on ACT straight out
#     of PSUM, rowsum via a ones-column appended to V in the PV matmul,
#     one reciprocal + multiply at the end.
import numpy as np

D_MODEL = 768
NUM_HEADS = 12
DK = 64
B = 2
S_FULL = 4096
N_CORES = 8
HPC = 3  # heads per core
CT = D_MODEL // 128  # contraction tiles for projections


def _chunk_sizes(ktiles):
    # 3 k-tiles per exp chunk; two independent streams each own a 3-bank
    # psum slot + a 1-bank output accumulator (3+3+1+1 = 8 banks)
    out = []
    rem = ktiles
    if rem % 3:
        out.append(rem % 3)
        rem -= rem % 3
    while rem > 0:
        out.append(3)
        rem -= 3
    return out


def _emit(nc, tc, S):
    import concourse.mybir as mybir
    from contextlib import ExitStack

    f32 = mybir.dt.float32
    fr = mybir.dt.float16
    Exp = mybir.ActivationFunctionType.Exp
    ADD = mybir.AluOpType.add

    QB = S // 512  # 512-query blocks
    ST = S // 128  # 128-row tiles of S (also k-tiles)
    CHUNKS = _chunk_sizes(ST)

    # ---- DRAM I/O ----
    xq = nc.dram_tensor("xq_t", [D_MODEL, S], fr, kind="ExternalInput")
    xk = nc.dram_tensor("xk_t", [D_MODEL, S], fr, kind="ExternalInput")
    xv = nc.dram_tensor("xv_t", [D_MODEL, S], fr, kind="ExternalInput")
    wq = nc.dram_tensor("wq_t", [D_MODEL, 256], fr, kind="ExternalInput")
    wk = nc.dram_tensor("wk_t", [D_MODEL, 256], fr, kind="ExternalInput")
    wv = nc.dram_tensor("wv_t", [D_MODEL, 256], fr, kind="ExternalInput")
    wo = nc.dram_tensor("wo_t", [DK, HPC, D_MODEL], fr, kind="ExternalInput")
    bqd = nc.dram_tensor("bq_p", [128, 2], f32, kind="ExternalInput")
    bkd = nc.dram_tensor("bk_p", [128, 2], f32, kind="ExternalInput")
    bvd = nc.dram_tensor("bv_p", [128, HPC * DK], f32, kind="ExternalInput")
    y_out = nc.dram_tensor("y_out", [S, D_MODEL], f32, kind="ExternalOutput")

    ctx = ExitStack()
    with ctx:
        persist = ctx.enter_context(tc.tile_pool(name="persist", bufs=1))
        xpool = ctx.enter_context(tc.tile_pool(name="xpool", bufs=4))
        ptpool = ctx.enter_context(tc.tile_pool(name="ptpool", bufs=2))
        spool = ctx.enter_context(tc.tile_pool(name="spool", bufs=2))
        ps = ctx.enter_context(tc.tile_pool(name="ps", bufs=1, space="PSUM"))

        def s_slot(i):
            return ps.tile([128, 1536], f32, tag=("s3a" if i % 2 == 0 else "s3b"),
                           name=f"sslot{i % 2}")

        def o_slot(i):
            return ps.tile([128, 512], f32, tag=("oa" if i % 2 == 0 else "ob"),
                           name=f"oslot{i % 2}")

        # ---- persistent SBUF ----
        wq_sb = persist.tile([128, CT, 256], fr, tag="wq_sb")
        wk_sb = persist.tile([128, CT, 256], fr, tag="wk_sb")
        wv_sb = persist.tile([128, CT, 256], fr, tag="wv_sb")
        wo_sb = persist.tile([DK, HPC, D_MODEL], fr, tag="wo_sb")
        bq_sb = persist.tile([128, 2], f32, tag="bq_sb")
        bk_sb = persist.tile([128, 2], f32, tag="bk_sb")
        bv_sb = persist.tile([128, HPC * DK], f32, tag="bv_sb")
        ones_sb = persist.tile([128, DK], fr, tag="ones_sb")
        qt01 = persist.tile([128, S], fr, tag="qt01")
        qt2 = persist.tile([128, S], fr, tag="qt2")
        kt01 = persist.tile([128, S], fr, tag="kt01")
        kt2 = persist.tile([128, S], fr, tag="kt2")
        v_all = persist.tile([128, ST, HPC, DK + 1], fr, tag="v_all")
        ot = [
            persist.tile([DK + 1, S], fr, tag=f"ot{h}", name=f"ot{h}")
            for h in range(HPC)
        ]

        nc.sync.dma_start(wq_sb[:], wq[:].rearrange("(o p) m -> p o m", p=128))
        nc.sync.dma_start(wk_sb[:], wk[:].rearrange("(o p) m -> p o m", p=128))
        nc.sync.dma_start(wv_sb[:], wv[:].rearrange("(o p) m -> p o m", p=128))
        nc.sync.dma_start(wo_sb[:], wo[:])
        nc.sync.dma_start(bq_sb[:], bqd[:])
        nc.sync.dma_start(bk_sb[:], bkd[:])
        nc.sync.dma_start(bv_sb[:], bvd[:])
        nc.vector.memset(ones_sb[:], 1.0)
        nc.vector.memset(v_all[:, :, :, DK : DK + 1], 1.0)

        # ---- q/k projections (transposed form [heads*64, S]) ----
        def proj_qk_block(x_dram, w_sb, b_sb, dst01, dst2, qb, xtag):
            sl = slice(qb * 512, (qb + 1) * 512)
            xt = xpool.tile([128, CT, 512], fr, tag=xtag, name=f"xt_{xtag}")
            nc.sync.dma_start(
                xt[:], x_dram[:, sl].rearrange("(o p) s -> p o s", p=128)
            )
            slot = s_slot(qb)
            p1 = slot[:, 0:512]
            p2 = slot[:, 512:1024]
            for c in range(CT):
                nc.tensor.matmul(
                    p1, w_sb[:, c, 0:128], xt[:, c, :],
                    start=(c == 0), stop=(c == CT - 1),
                )
                nc.tensor.matmul(
                    p2, w_sb[:, c, 128:256], xt[:, c, :],
                    start=(c == 0), stop=(c == CT - 1),
                )
            nc.vector.tensor_scalar(dst01[:, sl], p1, b_sb[:, 0:1], None, ADD)
            nc.vector.tensor_scalar(dst2[:, sl], p2, b_sb[:, 1:2], None, ADD)

        # order: k first, then v, then q
        for qb in range(QB):
            proj_qk_block(xk, wk_sb, bk_sb, kt01, kt2, qb, "xk")

        # ---- v projection (natural layout [S, 64] per head) ----
        for g in range(ST // 4):
            gsl = slice(g * 512, (g + 1) * 512)
            xt = xpool.tile([128, CT, 512], fr, tag="xv")
            nc.sync.dma_start(
                xt[:], xv[:, gsl].rearrange("(o p) s -> p o s", p=128)
            )
            for st in range(g * 4, g * 4 + 4):
                off = (st % 4) * 128
                pv = s_slot(st)[:, 0:256]
                for c in range(CT):
                    nc.tensor.matmul(
                        pv, xt[:, c, off : off + 128], wv_sb[:, c, 0:256],
                        start=(c == 0), stop=(c == CT - 1),
                    )
                for h in range(HPC):
                    nc.vector.tensor_add(
                        v_all[:, st, h, 0:DK],
                        pv[:, h * DK : (h + 1) * DK],
                        bv_sb[:, h * DK : (h + 1) * DK],
                    )


        for qb in range(QB):
            proj_qk_block(xq, wq_sb, bq_sb, qt01, qt2, qb, "xq")

        # ---- attention: paired streams, QK packed as concurrent row-groups ----
        # pair (h0,qb)+(h1,qb): h0 on array rows 0-63, h1 on rows 64-127
        # pair (h2,qb)+(h2,qb'): uses qt2/kt2 whose rows 64-127 duplicate h2
        def unit_aps(h, lane):
            rows = slice(0, DK) if lane == 0 else slice(DK, 128)
            if h < 2:
                return (qt01[rows, :], kt01[rows, :])
            return (qt2[rows, :], kt2[rows, :])

        def unit_state(h, qb, idx, lane):
            qt_ap, kt_ap = unit_aps(h, lane)
            return {
                "h": h, "sl": slice(qb * 512, (qb + 1) * 512),
                "po": o_slot(idx), "kk": 0, "qt": qt_ap, "kt": kt_ap,
            }

        def emit_chunk_qk(p_s, st_, j):
            kk = st_["kk"]
            kt_sl = slice((kk + j) * 128, (kk + j + 1) * 128)
            nc.tensor.matmul(
                p_s[:, j * 512 : (j + 1) * 512],
                st_["kt"][:, kt_sl], st_["qt"][:, st_["sl"]],
                start=True, stop=True,
            )

        def emit_chunk_act(p_s, idx, st_, cs):
            pt = ptpool.tile([128, 1536], fr, tag=f"pt{idx % 2}", name=f"pt{idx % 2}")
            nc.scalar.activation(pt[:, : cs * 512], p_s[:, : cs * 512], Exp, scale=0.125)
            st_["pv_pend"] = (pt, st_["kk"], cs)
            st_["kk"] += cs

        def emit_pv(st_):
            if st_.get("pv_pend") is None:
                return
            pt, kk, cs = st_["pv_pend"]
            h, po = st_["h"], st_["po"]
            for j in range(cs):
                nc.tensor.matmul(
                    po[0 : DK + 1, :],
                    v_all[:, kk + j, h, :],
                    pt[:, j * 512 : (j + 1) * 512],
                    start=(kk + j == 0), stop=(kk + j == ST - 1),
                )
            st_["pv_pend"] = None

        def finish_unit(idx, st_):
            h, sl, po = st_["h"], st_["sl"], st_["po"]
            nc.vector.tensor_copy(ot[h][0 : DK + 1, sl], po[0 : DK + 1, :])
            rs_row = spool.tile([1, 512], fr, tag="rsrow")
            nc.sync.dma_start(rs_row[:], ot[h][DK : DK + 1, sl])
            rbc = spool.tile([DK, 512], fr, tag="rbc")
            nc.gpsimd.partition_broadcast(rbc[:], rs_row[0:1, :])
            rsb = spool.tile([DK, 512], f32, tag="rsb", bufs=3)
            nc.vector.reciprocal(rsb[:], rbc[:])
            nc.vector.tensor_mul(ot[h][0:DK, sl], ot[h][0:DK, sl], rsb[:])

        pairs = [((0, qb), (1, qb)) for qb in range(QB)]
        h2qbs = list(range(QB))
        while len(h2qbs) >= 2:
            pairs.append(((2, h2qbs.pop(0)), (2, h2qbs.pop(0))))
        solo = [(2, qb) for qb in h2qbs]

        def emit_y(qts):
            for qt in qts:
                q_sl = slice(qt * 128, (qt + 1) * 128)
                py = s_slot(qt)[:, 0:768]
                for h in range(HPC):
                    nc.tensor.matmul(
                        py[:, 0:512], ot[h][0:DK, q_sl], wo_sb[:, h, 0:512],
                        start=(h == 0), stop=(h == HPC - 1),
                    )
                    nc.tensor.matmul(
                        py[:, 512:768], ot[h][0:DK, q_sl], wo_sb[:, h, 512:768],
                        start=(h == 0), stop=(h == HPC - 1),
                    )
                ysb = spool.tile([128, D_MODEL], f32, tag="ysb", bufs=3)
                nc.vector.tensor_copy(ysb[:], py)
                nc.sync.dma_start(y_out[q_sl, :], ysb[:])

        pending = None
        for pi, ((hA, qbA), (hB, qbB)) in enumerate(pairs):
            stA = unit_state(hA, qbA, 0, 0)
            stB = unit_state(hB, qbB, 1, 1)
            for ci, cs in enumerate(CHUNKS):
                psA = s_slot(0)
                psB = s_slot(1)
                for j in range(cs):
                    emit_chunk_qk(psA, stA, j)
                    emit_chunk_qk(psB, stB, j)
                emit_pv(stA)
                emit_pv(stB)
                emit_chunk_act(psA, 0, stA, cs)
                emit_chunk_act(psB, 1, stB, cs)
                if ci == 0 and pending is not None:
                    finish_unit(0, pending[0])
                    finish_unit(1, pending[1])
                    pending = None
            emit_pv(stA)
            emit_pv(stB)
            pending = (stA, stB)
        if pending is not None:
            finish_unit(0, pending[0])
            finish_unit(1, pending[1])
            pending = None
        for h, qb in solo:
            stA = unit_state(h, qb, 0, 0)
            for ci, cs in enumerate(CHUNKS):
                psA = s_slot(0)
                for j in range(cs):
                    emit_chunk_qk(psA, stA, j)
                emit_pv(stA)
                emit_chunk_act(psA, 0, stA, cs)
            emit_pv(stA)
            finish_unit(0, stA)

        # ---- output projection partials ----
        emit_y(range(ST))


def build_nc(S=S_FULL):
    import concourse.bacc as bacc
    import concourse.tile as tile

    nc = bacc.Bacc("TRN2", target_bir_lowering=False, debug=False)
    with tile.TileContext(nc) as tc:
        _emit(nc, tc, S)
    nc.compile()
    return nc


def make_in_maps(query, key, value, Wq, bq, Wk, bk, Wv, bv, Wo, bo, S=S_FULL):
    """Per-core input dicts (host-side sharding / layout marshalling)."""
    query = np.asarray(query, dtype=np.float32)
    key = np.asarray(key, dtype=np.float32)
    value = np.asarray(value, dtype=np.float32)
    Wq, Wk, Wv, Wo = (np.asarray(w, dtype=np.float32) for w in (Wq, Wk, Wv, Wo))
    bq, bk, bv = (np.asarray(x, dtype=np.float32) for x in (bq, bk, bv))

    xq_b = [np.ascontiguousarray(query[b].T.astype(np.float16)) for b in range(B)]
    xk_b = [np.ascontiguousarray(key[b].T.astype(np.float16)) for b in range(B)]
    xv_b = [np.ascontiguousarray(value[b].T.astype(np.float16)) for b in range(B)]
    WqT, WkT, WvT, WoT = (w.T.astype(np.float16) for w in (Wq, Wk, Wv, Wo))

    in_maps = []
    for core in range(N_CORES):
        b = core // 4
        h0 = HPC * (core % 4)
        cs = slice(h0 * DK, (h0 + HPC) * DK)
        bq_p = np.zeros((128, 2), np.float32)
        bk_p = np.zeros((128, 2), np.float32)
        bq_l, bk_l, bv_l = bq[cs], bk[cs], bv[cs]
        bq_p[:, 0], bq_p[0:DK, 1], bq_p[DK:128, 1] = (
            bq_l[0:128], bq_l[128:192], bq_l[128:192])
        bk_p[:, 0], bk_p[0:DK, 1], bk_p[DK:128, 1] = (
            bk_l[0:128], bk_l[128:192], bk_l[128:192])
        in_maps.append({
            "xq_t": xq_b[b],
            "xk_t": xk_b[b],
            "xv_t": xv_b[b],
            "wq_t": np.concatenate(
                [WqT[:, cs], WqT[:, cs.start + 2 * DK : cs.stop]], axis=1
            ),
            "wk_t": np.concatenate(
                [WkT[:, cs], WkT[:, cs.start + 2 * DK : cs.stop]], axis=1
            ),
            "wv_t": np.concatenate(
                [WvT[:, cs], np.zeros((D_MODEL, 256 - HPC * DK), np.float16)], axis=1
            ),
            "wo_t": np.ascontiguousarray(
                WoT[cs, :].reshape(HPC, DK, D_MODEL).transpose(1, 0, 2)
            ),
            "bq_p": bq_p,
            "bk_p": bk_p,
            "bv_p": np.tile(bv_l[None, :], (128, 1)).astype(np.float32),
        })
    return in_maps


_NC_CACHE = {}


def kernel(query, key, value, Wq, bq, Wk, bk, Wv, bv, Wo, bo):
    from concourse import bass_utils

    if S_FULL not in _NC_CACHE:
        _NC_CACHE[S_FULL] = build_nc(S_FULL)
    nc = _NC_CACHE[S_FULL]

    in_maps = make_in_maps(query, key, value, Wq, bq, Wk, bk, Wv, bv, Wo, bo)
    res = None
    for attempt in range(3):
        try:
            res = bass_utils.run_bass_kernel_spmd(
                nc, in_maps, core_ids=list(range(N_CORES))
            )
            break
        except Exception:
            if attempt == 2:
                raise

    bo = np.asarray(bo, dtype=np.float32)
    y = np.zeros((B, S_FULL, D_MODEL), np.float32)
    for core in range(N_CORES):
        y[core // 4] += np.asarray(res.results[core]["y_out"])
    y += bo[None, None, :]
    return y



# revision 16
# speedup vs baseline: 1.2475x; 1.2475x over previous
# Multi-head attention (B=2, S=4096, D=768, H=12) on 8 Trainium2 NeuronCores.
#
# Sharding: 24 (batch, head) units -> 3 heads x 1 batch per core.
#   core c: batch b = c // 4, heads h0..h0+2 where h0 = 3 * (c % 4).
# Each core computes q/k/v projections for its heads, attention, and a
# row-parallel partial of the output projection (its 192 columns of the
# concat dimension).  Host sums the 4 partials per batch and adds bo.
#
# v2 notes:
#   - softmax exp is split across TWO engines: stream-A chunks use the
#     scalar engine's exact Exp; stream-B chunks use a custom DVE op
#     (EXP2_BITS_ANT) that computes fp16 bit-patterns of 2^t via a
#     magic-add floor + quadratic mantissa correction (max rel err 0.26%,
#     constant factor cancels in softmax).  This halves the ACT-engine
#     serialization that paced the attention loop.
#   - the 1024*log2(e)/8 exp prescale is folded into Wq/Wk host-side so
#     both engines read raw psum scores with free scale/bias.
#   - shifted softmax: exp(s*0.125 - 4) everywhere (cancels; keeps the
#     custom-op int16 bits in range).
#   - q/k projection bias adds moved to the scalar engine ([P,1] bias).
#   - softmax normalization: reciprocal_approx_fast on the [1,512] rowsum
#     + partition-broadcast multiply (replaces 4us/unit full reciprocal).
#   - output projection packs h0+h1 into one 128-contraction matmul
#     (ot01 tile holds h0 dims on partitions 0-63, h1 on 64-127).
import numpy as np

D_MODEL = 768
NUM_HEADS = 12
DK = 64
B = 2
S_FULL = 4096
N_CORES = 8
HPC = 3  # heads per core
CT = D_MODEL // 128  # contraction tiles for projections

LOG2E = 1.4426950408889634
SHIFT = 4.0
FOLD = float(np.sqrt(1024.0 * LOG2E / 8.0))  # folded into Wq, Wk, bq, bk
# custom-op constants (see exp derivation): bits = z + c2*(frac^2 - K)
MAGIC = 1.5 * 2**33
C1_CONST = 1024.0 * 22.0 + 512.0 - SHIFT * 1024.0 * LOG2E
C2_POLY = 0.0003320625
K_CONST = 1788398.2683982684
DVE_GAIN = 128.4978  # measured constant factor of the custom op output
ACT_SCALE = 0.125 * 8.0 / (1024.0 * LOG2E)  # psum -> exp arg (s_raw*0.125)
ACT_BIAS = float(-SHIFT + np.log(DVE_GAIN))  # match DVE gain so chunks mix


def _register_exp2():
    import numpy as np
    from concourse.dve_spec import Spec, Src0, C0, C1, C2, C3, sq, _spill_c3_to_src1
    from concourse import dve_ops
    from concourse.dve_ops import DveOp

    for o in dve_ops.OPS:
        if o.name == "EXP2_BITS_ANT":
            return o

    def _ref(in0, in1, s0, s1, imm2):
        f32 = np.float32
        zz = in0.astype(f32) + f32(imm2)
        aa = zz + f32(s0)
        ii = aa - f32(s0)
        ff = zz - ii
        K = in1.reshape(in1.shape[0], -1).astype(f32)[:, :1]
        return (zz + (ff * ff - K) * f32(s1)).astype(f32)

    z = Src0 + C2
    f = z - ((z + C0) - C0)
    op = DveOp(
        "EXP2_BITS_ANT",
        Spec(body=_spill_c3_to_src1(z + (sq(f) - C3) * C1), reference=_ref),
        subdim=False,
        uops_sha={"v3": "9cb488f62a1208a8", "v4": "b09c3b7f02de2127"},
    )
    dve_ops.OPS.append(op)
    # borrow an unused production row (firmware-proven); this kernel never
    # emits CODY_WAITE_CASCADE so the per-NEFF uop table has no collision.
    dve_ops._SUB_OPCODE_FOR_NAME[op.name] = dve_ops.get_dve_sub_opcode(
        "CODY_WAITE_CASCADE"
    )
    dve_ops.CUSTOM_DVE_SPECS[op.name] = op.spec
    return op


def _chunk_sizes(ktiles):
    # 3 k-tiles per exp chunk; two independent streams each own a 3-bank
    # psum slot + a 1-bank output accumulator (3+3+1+1 = 8 banks)
    out = []
    rem = ktiles
    if rem % 3:
        out.append(rem % 3)
        rem -= rem % 3
    while rem > 0:
        out.append(3)
        rem -= 3
    return out


def _emit(nc, tc, S, debug_taps=False):
    import concourse.mybir as mybir
    from contextlib import ExitStack

    exp2_op = _register_exp2()

    f32 = mybir.dt.float32
    fr = mybir.dt.float16
    i16 = mybir.dt.int16
    Exp = mybir.ActivationFunctionType.Exp

    QB = S // 512  # 512-query blocks
    ST = S // 128  # 128-row tiles of S (also k-tiles)
    CHUNKS = _chunk_sizes(ST)

    # ---- DRAM I/O ----
    xq = nc.dram_tensor("xq_t", [D_MODEL, S], fr, kind="ExternalInput")
    xk = nc.dram_tensor("xk_t", [D_MODEL, S], fr, kind="ExternalInput")
    xv = nc.dram_tensor("xv_t", [D_MODEL, S], fr, kind="ExternalInput")
    wq = nc.dram_tensor("wq_t", [D_MODEL, 256], fr, kind="ExternalInput")
    wk = nc.dram_tensor("wk_t", [D_MODEL, 256], fr, kind="ExternalInput")
    wv = nc.dram_tensor("wv_t", [D_MODEL, 256], fr, kind="ExternalInput")
    wo01 = nc.dram_tensor("wo01_t", [128, D_MODEL], fr, kind="ExternalInput")
    wo2 = nc.dram_tensor("wo2_t", [DK, D_MODEL], fr, kind="ExternalInput")
    bqd = nc.dram_tensor("bq_p", [128, 2], f32, kind="ExternalInput")
    bkd = nc.dram_tensor("bk_p", [128, 2], f32, kind="ExternalInput")
    bvd = nc.dram_tensor("bv_p", [128, HPC * DK], f32, kind="ExternalInput")
    y_out = nc.dram_tensor("y_out", [S, D_MODEL], f32, kind="ExternalOutput")

    ctx = ExitStack()
    with ctx:
        persist = ctx.enter_context(tc.tile_pool(name="persist", bufs=1))
        xpool = ctx.enter_context(tc.tile_pool(name="xpool", bufs=4))
        ptpool = ctx.enter_context(tc.tile_pool(name="ptpool", bufs=2))
        spool = ctx.enter_context(tc.tile_pool(name="spool", bufs=2))
        ps = ctx.enter_context(tc.tile_pool(name="ps", bufs=1, space="PSUM"))

        def s_slot(i):
            return ps.tile([128, 1536], f32, tag=("s3a" if i % 2 == 0 else "s3b"),
                           name=f"sslot{i % 2}")

        def o_slot(i):
            return ps.tile([128, 512], f32, tag=("oa" if i % 2 == 0 else "ob"),
                           name=f"oslot{i % 2}")

        # ---- persistent SBUF ----
        wq_sb = persist.tile([128, CT, 256], fr, tag="wq_sb")
        wk_sb = persist.tile([128, CT, 256], fr, tag="wk_sb")
        wv_sb = persist.tile([128, CT, 256], fr, tag="wv_sb")
        wo01_sb = persist.tile([128, D_MODEL], fr, tag="wo01_sb")
        wo2_sb = persist.tile([DK, D_MODEL], fr, tag="wo2_sb")
        bq_sb = persist.tile([128, 2], f32, tag="bq_sb")
        bk_sb = persist.tile([128, 2], f32, tag="bk_sb")
        bv_sb = persist.tile([128, HPC * DK], f32, tag="bv_sb")
        kconst = persist.tile([128, 1], f32, tag="kconst")
        actb = persist.tile([128, 1], f32, tag="actb")
        qt01 = persist.tile([128, S], fr, tag="qt01")
        qt2 = persist.tile([128, S], fr, tag="qt2")
        kt01 = persist.tile([128, S], fr, tag="kt01")
        kt2 = persist.tile([128, S], fr, tag="kt2")
        v_all = persist.tile([128, ST, HPC, DK + 1], fr, tag="v_all")
        # ot01: h0 output dims on partitions 0-63, h1 on 64-127
        ot01 = persist.tile([128, S], fr, tag="ot01")
        # ot2: h2 dims on partitions 0-63; 64-127 zeroed (padded contraction)
        ot2 = persist.tile([128, S], fr, tag="ot2")

        nc.sync.dma_start(wq_sb[:], wq[:].rearrange("(o p) m -> p o m", p=128))
        nc.sync.dma_start(wk_sb[:], wk[:].rearrange("(o p) m -> p o m", p=128))
        nc.sync.dma_start(wv_sb[:], wv[:].rearrange("(o p) m -> p o m", p=128))
        nc.sync.dma_start(wo01_sb[:], wo01[:])
        nc.sync.dma_start(wo2_sb[:], wo2[:])
        nc.sync.dma_start(bq_sb[:], bqd[:])
        nc.sync.dma_start(bk_sb[:], bkd[:])
        nc.sync.dma_start(bv_sb[:], bvd[:])
        nc.vector.memset(kconst[:], K_CONST)
        nc.vector.memset(actb[:], ACT_BIAS)
        nc.vector.memset(v_all[:, :, :, DK : DK + 1], 1.0)
        nc.vector.memset(ot2[DK:128, :], 0.0)

        # ---- q/k projections (transposed form [heads*64, S]) ----
        def proj_qk_block(x_dram, w_sb, b_sb, dst01, dst2, qb, xtag):
            sl = slice(qb * 512, (qb + 1) * 512)
            xt = xpool.tile([128, CT, 512], fr, tag=xtag, name=f"xt_{xtag}")
            nc.sync.dma_start(
                xt[:], x_dram[:, sl].rearrange("(o p) s -> p o s", p=128)
            )
            slot = s_slot(qb)
            p1 = slot[:, 0:512]
            p2 = slot[:, 512:1024]
            for c in range(CT):
                nc.tensor.matmul(
                    p1, w_sb[:, c, 0:128], xt[:, c, :],
                    start=(c == 0), stop=(c == CT - 1),
                )
                nc.tensor.matmul(
                    p2, w_sb[:, c, 128:256], xt[:, c, :],
                    start=(c == 0), stop=(c == CT - 1),
                )
            # bias add on the scalar engine ([P,1] bias), psum -> sbuf fp16
            nc.scalar.add(dst01[:, sl], p1, b_sb[:, 0:1])
            nc.scalar.add(dst2[:, sl], p2, b_sb[:, 1:2])

        # order: k first, then v, then q
        for qb in range(QB):
            proj_qk_block(xk, wk_sb, bk_sb, kt01, kt2, qb, "xk")

        # ---- v projection (natural layout [S, 64] per head) ----
        for g in range(ST // 4):
            gsl = slice(g * 512, (g + 1) * 512)
            xt = xpool.tile([128, CT, 512], fr, tag="xv")
            nc.sync.dma_start(
                xt[:], xv[:, gsl].rearrange("(o p) s -> p o s", p=128)
            )
            for st in range(g * 4, g * 4 + 4):
                off = (st % 4) * 128
                pv = s_slot(st)[:, 0:256]
                for c in range(CT):
                    nc.tensor.matmul(
                        pv, xt[:, c, off : off + 128], wv_sb[:, c, 0:256],
                        start=(c == 0), stop=(c == CT - 1),
                    )
                nc.vector.tensor_add(
                    v_all[:, st, :, 0:DK],
                    pv[:, 0 : HPC * DK].rearrange("p (h d) -> p h d", h=HPC),
                    bv_sb[:].rearrange("p (h d) -> p h d", h=HPC),
                )

        for qb in range(QB):
            proj_qk_block(xq, wq_sb, bq_sb, qt01, qt2, qb, "xq")

        # ---- attention: paired streams, QK packed as concurrent row-groups ----
        # pair (h0,qb)+(h1,qb): h0 on array rows 0-63, h1 on rows 64-127
        # pair (h2,qb)+(h2,qb'): uses qt2/kt2 whose rows 64-127 duplicate h2
        def unit_aps(h, lane):
            rows = slice(0, DK) if lane == 0 else slice(DK, 128)
            if h < 2:
                return (qt01[rows, :], kt01[rows, :])
            return (qt2[rows, :], kt2[rows, :])

        def unit_state(h, qb, idx, lane):
            qt_ap, kt_ap = unit_aps(h, lane)
            return {
                "h": h, "sl": slice(qb * 512, (qb + 1) * 512),
                "po": o_slot(idx), "kk": 0, "qt": qt_ap, "kt": kt_ap,
                "lane": lane,
            }

        def emit_chunk_qk(p_s, st_, j):
            kk = st_["kk"]
            kt_sl = slice((kk + j) * 128, (kk + j + 1) * 128)
            nc.tensor.matmul(
                p_s[:, j * 512 : (j + 1) * 512],
                st_["kt"][:, kt_sl], st_["qt"][:, st_["sl"]],
                start=True, stop=True,
            )

        def emit_chunk_act(p_s, idx, st_, cs):
            pt = ptpool.tile([128, 1536], fr, tag=f"pt{idx % 2}", name=f"pt{idx % 2}")
            if st_["lane"] == 0:
                nc.scalar.activation(
                    pt[:, : cs * 512], p_s[:, : cs * 512], Exp,
                    scale=ACT_SCALE, bias=actb[:, 0:1],
                )
            else:
                nc.vector._custom_dve(
                    exp2_op,
                    out=pt[:, : cs * 512].bitcast(i16),
                    in0=p_s[:, : cs * 512],
                    in1=kconst[:],
                    s0=MAGIC, s1=C2_POLY, imm2=C1_CONST,
                )
            if debug_taps and st_["kk"] == 0 and idx == 0 and not dbg_pt_done[0]:
                dbg_pt_done[0] = True
                nc.sync.dma_start(dbg_pt[:], pt[:, 0:512])
            st_["pv_pend"] = (pt, st_["kk"], cs)
            st_["kk"] += cs

        def emit_pv(st_):
            if st_.get("pv_pend") is None:
                return
            pt, kk, cs = st_["pv_pend"]
            h, po = st_["h"], st_["po"]
            for j in range(cs):
                nc.tensor.matmul(
                    po[0 : DK + 1, :],
                    v_all[:, kk + j, h, :],
                    pt[:, j * 512 : (j + 1) * 512],
                    start=(kk + j == 0), stop=(kk + j == ST - 1),
                )
            st_["pv_pend"] = None

        if debug_taps:
            dbg_rs = nc.dram_tensor("dbg_rs", [1, 512], f32, kind="ExternalOutput")
            dbg_rbc = nc.dram_tensor("dbg_rbc", [DK, 512], f32, kind="ExternalOutput")
            dbg_po = nc.dram_tensor("dbg_po", [DK + 1, 512], f32, kind="ExternalOutput")
            dbg_pt = nc.dram_tensor("dbg_pt", [128, 512], fr, kind="ExternalOutput")
            dbg_done = [False]
            dbg_pt_done = [False]

        def finish_unit(idx, st_):
            h, sl, po = st_["h"], st_["sl"], st_["po"]
            if debug_taps and not dbg_done[0] and h == 0:
                dbg_done[0] = True
                posb = spool.tile([DK + 1, 512], f32, tag="dbgpo", name="posb")
                nc.vector.tensor_copy(posb[:], po[0 : DK + 1, :])
                nc.sync.dma_start(dbg_po[:], posb[:])
            # normalization factors from the rowsum row of the PV accumulator
            # rowsum row: lane-64 copy to SBUF, DMA-relocate to partition 0,
            # reciprocal there, then broadcast (DVE lanes are partition-locked).
            rs_hold = spool.tile([DK + 1, 512], f32, tag="rshold", bufs=3)
            nc.vector.tensor_copy(rs_hold[DK : DK + 1, :], po[DK : DK + 1, :])
            rs0 = spool.tile([1, 512], f32, tag="rs0", bufs=3)
            nc.sync.dma_start(rs0[:], rs_hold[DK : DK + 1, :])
            rs = spool.tile([1, 512], f32, tag="rs", bufs=3)
            nc.vector.reciprocal_approx_fast(rs[:], rs0[:])
            rbc = spool.tile([DK, 512], f32, tag="rbc", bufs=3)
            nc.gpsimd.partition_broadcast(rbc[:], rs[0:1, :])
            if debug_taps and dbg_done[0] and h == 0 and dbg_done[0] != 2:
                dbg_done[0] = 2
                nc.sync.dma_start(dbg_rs[:], rs[:])
                nc.sync.dma_start(dbg_rbc[:], rbc[:])
            if h == 0:
                dst = ot01[0:DK, sl]
            elif h == 1:
                # DVE lanes are partition-locked; normalize at base 0 then
                # DMA-relocate to partitions 64-127 of ot01.
                fin1 = spool.tile([DK, 512], fr, tag="fin1", bufs=3, name="fin1")
                dst = fin1[:]
            else:
                dst = ot2[0:DK, sl]
            nc.vector.tensor_copy(dst, po[0:DK, :])
            nc.vector.tensor_mul(dst, dst, rbc[:])
            if h == 1:
                nc.sync.dma_start(ot01[DK:128, sl], dst)

        pairs = [((0, qb), (1, qb)) for qb in range(QB)]
        h2qbs = list(range(QB))
        while len(h2qbs) >= 2:
            pairs.append(((2, h2qbs.pop(0)), (2, h2qbs.pop(0))))
        solo = [(2, qb) for qb in h2qbs]

        def emit_y(qts):
            for qt in qts:
                q_sl = slice(qt * 128, (qt + 1) * 128)
                py = s_slot(qt)[:, 0:768]
                for part in range(2):
                    csl = slice(part * 512, 512 + part * 256)
                    nc.tensor.matmul(
                        py[:, csl], ot01[:, q_sl], wo01_sb[:, csl],
                        start=True, stop=False,
                    )
                    nc.tensor.matmul(
                        py[:, csl], ot2[:, q_sl],
                        wo2_pad[:, csl],
                        start=False, stop=True,
                    )
                ysb = spool.tile([128, D_MODEL], f32, tag="ysb", bufs=3)
                nc.scalar.copy(ysb[:], py)
                nc.sync.dma_start(y_out[q_sl, :], ysb[:])

        # zero-padded wo2 so the h2 matmul keeps the 128-contraction mode
        wo2_pad = persist.tile([128, D_MODEL], fr, tag="wo2_pad")
        nc.vector.memset(wo2_pad[DK:128, :], 0.0)
        nc.vector.tensor_copy(wo2_pad[0:DK, :], wo2_sb[:])

        pending = None
        for pi, ((hA, qbA), (hB, qbB)) in enumerate(pairs):
            stA = unit_state(hA, qbA, 0, 0)
            stB = unit_state(hB, qbB, 1, 1)
            for ci, cs in enumerate(CHUNKS):
                psA = s_slot(0)
                psB = s_slot(1)
                for j in range(cs):
                    emit_chunk_qk(psA, stA, j)
                    emit_chunk_qk(psB, stB, j)
                emit_pv(stA)
                emit_pv(stB)
                emit_chunk_act(psA, 0, stA, cs)
                emit_chunk_act(psB, 1, stB, cs)
                if ci == 0 and pending is not None:
                    finish_unit(0, pending[0])
                    finish_unit(1, pending[1])
                    pending = None
            emit_pv(stA)
            emit_pv(stB)
            pending = (stA, stB)
        if pending is not None:
            finish_unit(0, pending[0])
            finish_unit(1, pending[1])
            pending = None
        for h, qb in solo:
            stA = unit_state(h, qb, 0, 0)
            for ci, cs in enumerate(CHUNKS):
                psA = s_slot(0)
                for j in range(cs):
                    emit_chunk_qk(psA, stA, j)
                emit_pv(stA)
                emit_chunk_act(psA, 0, stA, cs)
            emit_pv(stA)
            finish_unit(0, stA)

        # ---- output projection partials ----
        emit_y(range(ST))


def build_nc(S=S_FULL, debug_taps=False):
    import concourse.bacc as bacc
    import concourse.tile as tile

    nc = bacc.Bacc("TRN2", target_bir_lowering=False, debug=False)
    with tile.TileContext(nc) as tc:
        _emit(nc, tc, S, debug_taps=debug_taps)
    nc.compile()
    return nc


def make_in_maps(query, key, value, Wq, bq, Wk, bk, Wv, bv, Wo, bo, S=S_FULL):
    """Per-core input dicts (host-side sharding / layout marshalling)."""
    query = np.asarray(query, dtype=np.float32)
    key = np.asarray(key, dtype=np.float32)
    value = np.asarray(value, dtype=np.float32)
    Wq, Wk, Wv, Wo = (np.asarray(w, dtype=np.float32) for w in (Wq, Wk, Wv, Wo))
    bq, bk, bv = (np.asarray(x, dtype=np.float32) for x in (bq, bk, bv))

    xq_b = [np.ascontiguousarray(query[b].T.astype(np.float16)) for b in range(B)]
    xk_b = [np.ascontiguousarray(key[b].T.astype(np.float16)) for b in range(B)]
    xv_b = [np.ascontiguousarray(value[b].T.astype(np.float16)) for b in range(B)]
    # fold the exp prescale into the q/k projections
    WqT = (Wq.T * FOLD).astype(np.float16)
    WkT = (Wk.T * FOLD).astype(np.float16)
    WvT = Wv.T.astype(np.float16)
    WoT = Wo.T.astype(np.float16)
    bqf = bq * FOLD
    bkf = bk * FOLD

    in_maps = []
    for core in range(N_CORES):
        b = core // 4
        h0 = HPC * (core % 4)
        cs = slice(h0 * DK, (h0 + HPC) * DK)
        bq_p = np.zeros((128, 2), np.float32)
        bk_p = np.zeros((128, 2), np.float32)
        bq_l, bk_l, bv_l = bqf[cs], bkf[cs], bv[cs]
        bq_p[:, 0], bq_p[0:DK, 1], bq_p[DK:128, 1] = (
            bq_l[0:128], bq_l[128:192], bq_l[128:192])
        bk_p[:, 0], bk_p[0:DK, 1], bk_p[DK:128, 1] = (
            bk_l[0:128], bk_l[128:192], bk_l[128:192])
        # wo01: rows 0-63 = h0 block, rows 64-127 = h1 block; wo2 = h2 block
        wo_blocks = WoT[cs, :].reshape(HPC, DK, D_MODEL)
        in_maps.append({
            "xq_t": xq_b[b],
            "xk_t": xk_b[b],
            "xv_t": xv_b[b],
            "wq_t": np.concatenate(
                [WqT[:, cs], WqT[:, cs.start + 2 * DK : cs.stop]], axis=1
            ),
            "wk_t": np.concatenate(
                [WkT[:, cs], WkT[:, cs.start + 2 * DK : cs.stop]], axis=1
            ),
            "wv_t": np.concatenate(
                [WvT[:, cs], np.zeros((D_MODEL, 256 - HPC * DK), np.float16)], axis=1
            ),
            "wo01_t": np.ascontiguousarray(
                wo_blocks[0:2].reshape(128, D_MODEL).astype(np.float16)
            ),
            "wo2_t": np.ascontiguousarray(wo_blocks[2].astype(np.float16)),
            "bq_p": bq_p,
            "bk_p": bk_p,
            "bv_p": np.tile(bv_l[None, :], (128, 1)).astype(np.float32),
        })
    return in_maps


_NC_CACHE = {}


def kernel(query, key, value, Wq, bq, Wk, bk, Wv, bv, Wo, bo):
    from concourse import bass_utils

    if S_FULL not in _NC_CACHE:
        _NC_CACHE[S_FULL] = build_nc(S_FULL)
    nc = _NC_CACHE[S_FULL]

    in_maps = make_in_maps(query, key, value, Wq, bq, Wk, bk, Wv, bv, Wo, bo)
    res = None
    for attempt in range(3):
        try:
            res = bass_utils.run_bass_kernel_spmd(
                nc, in_maps, core_ids=list(range(N_CORES))
            )
            break
        except Exception:
            if attempt == 2:
                raise
    bo = np.asarray(bo, dtype=np.float32)
    y = np.zeros((B, S_FULL, D_MODEL), np.float32)
    for core in range(N_CORES):
        y[core // 4] += np.asarray(res.results[core]["y_out"])
    y += bo[None, None, :]
    return y
